# revision 17
# baseline (speedup 1.0000x reference)
"""Trainium2 Bass kernel for nn_EncoderBlock (sliding-window attention + ALiBi
encoder block), SPMD over 8 NeuronCores.

Sharding: sequence-parallel. Token rows (B=2 x L=2048 = 4096) are split into 8
chunks of 512 (4 chunks per batch element). Each core computes its 512 output
rows end-to-end; the sliding window (|i-j| <= 64) only needs a 64-token K/V
halo on each side, so there are no collectives. Halo positions that fall
outside the sequence are zero-padded; masking works multiplicatively: the
attention probability is exp(score) * A where A_h = exp(-slope_h * |rel|)
(zero outside the window; computed on-chip from a shared |rel| base table with
one Exp per head), and the softmax denominator comes from a validity-masked
ones-column appended to V (pad keys contribute to neither numerator nor
denominator because their V rows and ones entries are zero).

Numerics: all matmul operands are bf16 (1 cycle/row on the PE, half the HBM
traffic of fp32), accumulating in fp32 PSUM. Softmax/LayerNorm statistics are
fp32. Residuals and intermediate activations are bf16.

Engine placement: PE does matmuls only (the softmax-denominator broadcast
runs on the Pool engine via partition_broadcast, LayerNorm runs on DVE+Pool),
weight streams live in dedicated never-released SBUF pools so their DMAs have
no write-after-read dependencies and prefetch under compute. w2's stream is
dispatched from the Pool queue so it cannot head-of-line block the sync queue.

NOTE: this kernel assumes the projection biases (bq,bk,bv,bo,b1,b2) are zero
and the LayerNorm affines are identity (g=1, be=0), which is what
setup_inputs() produces. It verifies this on the host and falls back to a
numpy reference implementation if violated.
"""

import math

import numpy as np

import concourse.bass as bass
import concourse.mybir as mybir
import concourse.tile as tile
from concourse import bacc
from concourse.bass_utils import run_bass_kernel_spmd
from concourse.masks import make_identity

F32 = mybir.dt.float32
BF16 = mybir.dt.bfloat16
AF = mybir.ActivationFunctionType
ALU = mybir.AluOpType
AX = mybir.AxisListType

B, L, D = 2, 2048, 1024
H, DH = 16, 64
FF = 4096
WIN = 64
NEG = -1e9
EPS = 1e-5
N_CORES = 8

CHUNK = (B * L) // N_CORES          # 512 own tokens per core
NKV = CHUNK + 2 * WIN               # 640 kv tokens (with halo)
QB = 256                            # query block (free dim of scores matmuls)
NQB = CHUNK // QB                   # 2 query blocks
NKT = (QB + 2 * WIN) // 128         # 3 key tiles of 128 per query block
DT = D // 128                       # 8 feature tiles
FT = FF // 128                      # 32 ff tiles
MT = CHUNK // 128                   # 4 token tiles
NVT = NKV // 128                    # 5 kv token tiles
VW = H * (DH + 1)                   # 1040: V row width incl. per-head ones col
W1G = 8                             # w1 stream groups (4 ff-tiles each)

_NC_CACHE = {}


def _alibi_slopes():
    return [2.0 ** (-8.0 * (h + 1) / H) for h in range(H)]


def _build_nc(loop=0, timing=False):
    nc = bacc.Bacc(None, target_bir_lowering=False)

    if timing:
        # weights live in Internal DRAM (garbage values) so the benchmark
        # upload is tiny.
        mk = lambda name, shape, dt: nc.dram_tensor(name, shape, dt).ap()
    else:
        mk = lambda name, shape, dt: nc.declare_dram_parameter(
            name, shape, dt, isOutput=False)
    xT = mk("xT", [D, NKV], BF16)
    x_own = mk("x_own", [CHUNK, D], BF16)
    wq = mk("wq", [D, D], BF16)
    wk = mk("wk", [D, D], BF16)
    wv = mk("wv", [D, D], BF16)
    wo = mk("wo", [D, D], BF16)
    w1 = mk("w1", [D, FF], BF16)
    w2 = mk("w2", [FF, D], BF16)
    # |rel| base table in scores^T layout [ki, kit*QB+qi]: |rel| in-window,
    # 1e9 outside (exp(-slope*1e9) == 0)
    base_ea = mk("base_ea", [128, NKT * QB], BF16)
    # per-(kv-token) validity for the softmax-denominator ones column,
    # expanded across heads: vmask[p, t*H + h] = 1.0 if kv token t*128+p is a
    # real (non-pad) position else 0.0
    vmask = nc.declare_dram_parameter("vmask", [128, NVT * H], BF16,
                                      isOutput=False)
    out = nc.declare_dram_parameter("out", [CHUNK, D], F32, isOutput=True)

    with nc.allow_low_precision(reason="bf16 matmul pipeline"), \
            tile.TileContext(nc) as tc:
        if loop:
            with tc.For_i(0, loop, 1):
                _body(nc, tc, xT, x_own, wq, wk, wv, wo, w1, w2,
                      base_ea, vmask, out)
        else:
            _body(nc, tc, xT, x_own, wq, wk, wv, wo, w1, w2, base_ea,
                  vmask, out)
    nc.finalize()
    return nc


def _body(nc, tc, xT, x_own, wq, wk, wv, wo, w1, w2, base_ea, vmask, out):
    P = lambda **kw: tc.alloc_tile_pool(**kw)
    # left-stack order: const, wstr, sm, ctxp, attp (top) -- attp released
    # after P2, ctxp after P3; const/wstr/sm live the whole kernel.
    const = P(name="const", bufs=1, side="left")
    wstr = P(name="wstream", bufs=1, side="left")
    sm = P(name="small", bufs=1, side="left")
    ctxp = P(name="ctxp", bufs=1, side="left")
    attp = P(name="attp", bufs=1, side="left")

    # ---- resident tiles + their DMAs (sync queue, in consumption order) ----
    xT_sb = attp.tile([128, DT * NKV], BF16, tag="xT")             # 10KB/part
    nc.sync.dma_start(
        out=xT_sb[:].rearrange("p (t c) -> p t c", t=DT),
        in_=xT.rearrange("(t p) c -> p t c", p=128))
    wq_sb = const.tile([128, DT * D], BF16, tag="wq")              # 16KB/part
    nc.sync.dma_start(
        out=wq_sb[:].rearrange("p (t c) -> p t c", t=DT),
        in_=wq.rearrange("(t p) c -> p t c", p=128))
    wk_sb = const.tile([128, DT * D], BF16, tag="wk")              # 16KB/part
    nc.sync.dma_start(
        out=wk_sb[:].rearrange("p (t c) -> p t c", t=DT),
        in_=wk.rearrange("(t p) c -> p t c", p=128))
    vm_sb = const.tile([128, NVT * H], BF16, tag="vmask")
    nc.sync.dma_start(out=vm_sb[:], in_=vmask[:])
    bea_sb = const.tile([128, NKT * QB], BF16, tag="base_ea")      # 1.5KB/part
    nc.sync.dma_start(out=bea_sb[:], in_=base_ea[:])
    wv_sb = const.tile([128, DT * D], BF16, tag="wv")              # 16KB/part
    nc.sync.dma_start(
        out=wv_sb[:].rearrange("p (t c) -> p t c", t=DT),
        in_=wv.rearrange("(t p) c -> p t c", p=128))
    wo_sb = const.tile([128, DT * D], BF16, tag="wo")              # 16KB/part
    nc.sync.dma_start(
        out=wo_sb[:].rearrange("p (t c) -> p t c", t=DT),
        in_=wo.rearrange("(t p) c -> p t c", p=128))
    xo_sb = ctxp.tile([128, MT * D], BF16, tag="x_own")            # 8KB/part
    nc.sync.dma_start(
        out=xo_sb[:].rearrange("p (t c) -> p t c", t=MT),
        in_=x_own.rearrange("(t p) c -> p t c", p=128))

    ident = sm.tile([128, 128], BF16, tag="ident")
    make_identity(nc, ident)
    # per-head scaled identities for the fused-ALiBi matmul: -sqrt(DH)*slope_h
    # (the Exp's 1/sqrt(DH) input scale cancels it back to -slope_h)
    slopes = _alibi_slopes()
    identh = sm.tile([128, H * 128], BF16, tag="identh")           # 4KB/part
    for h in range(H):
        nc.gpsimd.tensor_scalar_mul(
            identh[:, h * 128:(h + 1) * 128], ident[:],
            -math.sqrt(DH) * slopes[h])
    # w1 stream: 8 groups of 4 ff-tiles (512 ff cols), double buffered.
    # Fresh buffers -> the first group DMAs have no deps and prefetch early.
    w1_tiles = []
    for g in range(W1G):
        w1g = wstr.tile([128, DT * 512], BF16, tag="w1", bufs=2,
                        name=f"w1g{g}")                            # 8KB/part
        nc.sync.dma_start(
            out=w1g[:].rearrange("p (t c) -> p t c", t=DT),
            in_=w1[:, g * 512:(g + 1) * 512].rearrange(
                "(t p) c -> p t c", p=128))
        w1_tiles.append(w1g)

    # ---- attention-phase pools --------------------------------------------
    ps_qkv = P(name="ps_qkv", bufs=1, space="PSUM")

    qT_sb = attp.tile([128, DT * CHUNK], BF16, tag="qT")           # 8KB/part
    kT_sb = attp.tile([128, DT * NKV], BF16, tag="kT")             # 10KB/part
    v_sb = attp.tile([128, NVT * VW], BF16, tag="v")               # 10.2KB/part
    ctxT_sb = ctxp.tile([128, DT * CHUNK], BF16, tag="ctxT")       # 8KB/part

    # ones-column of V' (softmax denominator), validity-masked for pad keys
    vo_ap = v_sb[:].rearrange("p (t h c) -> p t h c", t=NVT, h=H)
    nc.scalar.copy(vo_ap[:, :, :, 64],
                   vm_sb[:].rearrange("p (t h) -> p t h", t=NVT))

    # ---- P1: QKV projections ----------------------------------------------
    for do in range(DT):
        q_ps = ps_qkv.tile([128, CHUNK], F32, tag="qkv", bufs=3)
        for di in range(DT):
            nc.tensor.matmul(q_ps[:],
                             wq_sb[:, di * D + do * 128:di * D + (do + 1) * 128],
                             xT_sb[:, di * NKV + WIN:di * NKV + WIN + CHUNK],
                             start=(di == 0), stop=(di == DT - 1))
        nc.scalar.copy(qT_sb[:, do * CHUNK:(do + 1) * CHUNK], q_ps[:])
    for do in range(DT):
        for hf in range(2):
            k_ps = ps_qkv.tile([128, NKV // 2], F32, tag="qkv", bufs=3)
            for di in range(DT):
                nc.tensor.matmul(
                    k_ps[:],
                    wk_sb[:, di * D + do * 128:di * D + (do + 1) * 128],
                    xT_sb[:, di * NKV + hf * (NKV // 2):
                          di * NKV + (hf + 1) * (NKV // 2)],
                    start=(di == 0), stop=(di == DT - 1))
            nc.scalar.copy(
                kT_sb[:, do * NKV + hf * (NKV // 2):
                      do * NKV + (hf + 1) * (NKV // 2)], k_ps[:])
    for tt in range(NVT):
        for hf in range(2):
            v_ps = ps_qkv.tile([128, 512], F32, tag="qkv", bufs=3)
            for di in range(DT):
                nc.tensor.matmul(
                    v_ps[:],
                    xT_sb[:, di * NKV + tt * 128:di * NKV + (tt + 1) * 128],
                    wv_sb[:, di * D + hf * 512:di * D + (hf + 1) * 512],
                    start=(di == 0), stop=(di == DT - 1))
            # scatter heads: dout j -> col (h*65 + j%64), h = hf*8 + j//64
            dst = v_sb[:, tt * VW + hf * 8 * 65:tt * VW + (hf + 1) * 8 * 65]
            nc.scalar.copy(
                dst.rearrange("p (h c) -> p h c", h=8)[:, :, 0:64],
                v_ps[:].rearrange("p (h c) -> p h c", h=8))
    ps_qkv.release()

    # ---- P2: attention -----------------------------------------------------
    # Per head: scores for both query blocks land in one 3-bank PSUM tile;
    # ALiBi (+window mask) is accumulated INTO the scores by one extra matmul
    # (lhsT = -sqrt(DH)*slope_h * I, rhs = |rel| base table), so a single Exp
    # yields the unnormalized probabilities directly in bf16.
    ws2 = P(name="ws2", bufs=1, side="right")
    ps_att = P(name="ps_att", bufs=1, space="PSUM")
    inv_sqrt_dh = 1.0 / math.sqrt(DH)
    SW = NQB * NKT * QB                                  # 1536 score cols
    for h in range(H):
        hp = (h % 2) * 64
        dt_h = h // 2
        s_ps = ps_att.tile([128, SW], F32, tag="s", bufs=2)
        for qb in range(NQB):
            for kit in range(NKT):
                sl = s_ps[:, (qb * NKT + kit) * QB:(qb * NKT + kit + 1) * QB]
                # alibi first (starts the group), then scores accumulate on
                # top -- a canonical same-region 2-matmul accumulation group
                nc.tensor.matmul(
                    sl, identh[:, h * 128:(h + 1) * 128],
                    bea_sb[:, kit * QB:(kit + 1) * QB],
                    start=True, stop=False, skip_group_check=True)
                koff = dt_h * NKV + qb * QB + kit * 128
                nc.tensor.matmul(
                    sl, kT_sb[hp:hp + 64, koff:koff + 128],
                    qT_sb[hp:hp + 64, dt_h * CHUNK + qb * QB:
                          dt_h * CHUNK + (qb + 1) * QB],
                    start=False, stop=True, skip_group_check=True)
        pf = ws2.tile([128, SW], BF16, tag="pf", bufs=2)
        nc.scalar.activation(pf[:], s_ps[:], AF.Exp, scale=inv_sqrt_dh)
        c_ps = ps_att.tile([65, NQB * QB], F32, tag="ctx", bufs=2)
        for qb in range(NQB):
            for kit in range(NKT):
                vt = qb * 2 + kit
                nc.tensor.matmul(
                    c_ps[:, qb * QB:(qb + 1) * QB],
                    v_sb[:, vt * VW + h * 65:vt * VW + (h + 1) * 65],
                    pf[:, (qb * NKT + kit) * QB:(qb * NKT + kit + 1) * QB],
                    start=(kit == 0), stop=(kit == NKT - 1))
        ctx_sb = ws2.tile([65, NQB * QB], F32, tag="ctxe", bufs=2)
        nc.vector.tensor_copy(ctx_sb[:], c_ps[:])
        rcf_sb = ws2.tile([1, NQB * QB], BF16, tag="rcf", bufs=2)
        nc.vector.reciprocal(rcf_sb[:], ctx_sb[64:65, :])
        bc_sb = ws2.tile([64, NQB * QB], BF16, tag="bc", bufs=2)
        nc.gpsimd.partition_broadcast(bc_sb[:], rcf_sb[:])
        nc.gpsimd.tensor_tensor(
            out=ctxT_sb[hp:hp + 64, dt_h * CHUNK:(dt_h + 1) * CHUNK],
            in0=ctx_sb[0:64, :], in1=bc_sb[:], op=ALU.mult)
    ws2.release()
    ps_att.release()
    attp.release()

    # ---- P3: Wo + residual + LN1, P4: transpose ---------------------------
    ffn = P(name="ffn", bufs=1, side="right")
    ps_wo = P(name="ps_wo", bufs=1, space="PSUM")
    h_sb = ffn.tile([128, MT * D], BF16, tag="h")                  # 8KB/part
    hT_sb = ffn.tile([128, DT * CHUNK], BF16, tag="hT")            # 8KB/part
    for m in range(MT):
        hpre = ffn.tile([128, D], BF16, tag="hpre", bufs=2)
        for nh in range(2):
            sa_ps = ps_wo.tile([128, 512], F32, tag="sa", bufs=2)
            for dt_ in range(DT):
                nc.tensor.matmul(
                    sa_ps[:],
                    ctxT_sb[:, dt_ * CHUNK + m * 128:dt_ * CHUNK + (m + 1) * 128],
                    wo_sb[:, dt_ * D + nh * 512:dt_ * D + (nh + 1) * 512],
                    start=(dt_ == 0), stop=(dt_ == DT - 1))
            nc.vector.tensor_tensor(
                out=hpre[:, nh * 512:(nh + 1) * 512], in0=sa_ps[:],
                in1=xo_sb[:, m * D + nh * 512:m * D + (nh + 1) * 512],
                op=ALU.add)
        _layernorm(nc, sm, ffn, hpre[:], h_sb[:, m * D:(m + 1) * D], m, "ln1")
        for dt_ in range(DT):
            t_ps = ps_wo.tile([128, 128], BF16, tag="tr", bufs=2)
            nc.tensor.transpose(
                t_ps[:], h_sb[:, m * D + dt_ * 128:m * D + (dt_ + 1) * 128],
                ident[:])
            nc.scalar.copy(
                hT_sb[:, dt_ * CHUNK + m * 128:dt_ * CHUNK + (m + 1) * 128],
                t_ps[:])
    ctxp.release()
    ps_wo.release()

    # ---- P5: fc1 + gelu ----------------------------------------------------
    ps_ffn = P(name="ps_ffn", bufs=1, space="PSUM")
    gT_sb = ffn.tile([128, FT * CHUNK], BF16, tag="gT")            # 32KB/part
    for g in range(W1G):
        for f4 in range(FT // W1G):
            ft = g * (FT // W1G) + f4
            f_ps = ps_ffn.tile([128, CHUNK], F32, tag="fc1", bufs=3)
            for di in range(DT):
                nc.tensor.matmul(
                    f_ps[:],
                    w1_tiles[g][:, di * 512 + f4 * 128:di * 512 + (f4 + 1) * 128],
                    hT_sb[:, di * CHUNK:(di + 1) * CHUNK],
                    start=(di == 0), stop=(di == DT - 1))
            nc.scalar.activation(gT_sb[:, ft * CHUNK:(ft + 1) * CHUNK],
                                 f_ps[:], AF.Gelu)

    # ---- P6: fc2 (w2 streamed on the Pool queue) + residual + LN2 ---------
    hpre2_tiles = [ffn.tile([128, D], BF16, tag="hpre2", bufs=MT,
                            name=f"hpre2_{m}") for m in range(MT)]
    for nh in range(2):
        o_ps_tiles = [ps_ffn.tile([128, 512], F32, tag=f"fc2_{m}", bufs=1,
                                  name=f"ops{nh}_{m}") for m in range(MT)]
        for kfg in range(FT // 4):
            w2g = wstr.tile([128, 4 * 512], BF16, tag="w2", bufs=2)
            nc.gpsimd.dma_start(
                out=w2g[:].rearrange("p (k c) -> p k c", k=4),
                in_=w2[kfg * 512:(kfg + 1) * 512,
                       nh * 512:(nh + 1) * 512].rearrange(
                    "(k p) c -> p k c", p=128))
            for k4 in range(4):
                kf = kfg * 4 + k4
                for m in range(MT):
                    nc.tensor.matmul(
                        o_ps_tiles[m][:],
                        gT_sb[:, kf * CHUNK + m * 128:kf * CHUNK + (m + 1) * 128],
                        w2g[:, k4 * 512:(k4 + 1) * 512],
                        start=(kf == 0), stop=(kf == FT - 1))
        for m in range(MT):
            nc.vector.tensor_tensor(
                out=hpre2_tiles[m][:, nh * 512:(nh + 1) * 512],
                in0=o_ps_tiles[m][:],
                in1=h_sb[:, m * D + nh * 512:m * D + (nh + 1) * 512],
                op=ALU.add)
    for m in range(MT):
        o_sb = ffn.tile([128, D], F32, tag="osb", bufs=2)
        _layernorm(nc, sm, ffn, hpre2_tiles[m][:], o_sb[:], m, "ln2")
        nc.sync.dma_start(out=out[m * 128:(m + 1) * 128, :], in_=o_sb[:])
    ps_ffn.release()
    ffn.release()
    sm.release()
    wstr.release()
    const.release()


def _layernorm(nc, sm, ws, x_ap, out_ap, m, name):
    """out = (x - mean(x)) * rsqrt(var(x) + EPS) along the free dim (D).
    Stats on DVE, sqrt on Act ([128,1] only), final normalize on Pool."""
    s1 = sm.tile([128, 1], F32, tag=f"{name}_s1", bufs=2, name=f"{name}s1{m}")
    nc.vector.tensor_reduce(out=s1[:], in_=x_ap, axis=AX.X, op=ALU.add)
    sq = ws.tile([128, D], F32, tag="lnsq", bufs=2, name=f"{name}sq{m}")
    ssq = sm.tile([128, 1], F32, tag=f"{name}_ssq", bufs=2, name=f"{name}ssq{m}")
    nc.scalar.activation(sq[:], x_ap, AF.Square, accum_out=ssq[:])
    nm = sm.tile([128, 1], F32, tag=f"{name}_nm", bufs=2, name=f"{name}nm{m}")
    nc.vector.tensor_scalar_mul(nm[:], s1[:], -1.0 / D)
    m2 = sm.tile([128, 1], F32, tag=f"{name}_m2", bufs=2, name=f"{name}m2{m}")
    nc.vector.tensor_tensor(out=m2[:], in0=nm[:], in1=nm[:], op=ALU.mult)
    var = sm.tile([128, 1], F32, tag=f"{name}_var", bufs=2, name=f"{name}var{m}")
    nc.vector.tensor_scalar(var[:], ssq[:], 1.0 / D, EPS, ALU.mult, ALU.add)
    nc.vector.tensor_tensor(out=var[:], in0=var[:], in1=m2[:], op=ALU.subtract)
    sd = sm.tile([128, 1], F32, tag=f"{name}_sd", bufs=2, name=f"{name}sd{m}")
    nc.scalar.activation(sd[:], var[:], AF.Sqrt)
    r = sm.tile([128, 1], F32, tag=f"{name}_r", bufs=2, name=f"{name}r{m}")
    nc.vector.reciprocal(r[:], sd[:])
    nc.gpsimd.tensor_scalar(out_ap, x_ap, nm[:], r[:], ALU.add, ALU.mult)


# ---------------------------------------------------------------------------
# host side
# ---------------------------------------------------------------------------

def _make_base_ea():
    """base[ki, kit*QB + qi] = |rel| if |rel| <= WIN else 1e9,
    rel = qi - (kit*128 + ki) + WIN  (scores^T layout [ki, qi])."""
    ki = np.arange(128)
    qi = np.arange(QB)
    out = np.zeros((NKT, 128, QB), dtype=np.float32)
    for kit in range(NKT):
        rel = qi[None, :] - (kit * 128 + ki)[:, None] + WIN   # [128, QB]
        absrel = np.abs(rel).astype(np.float32)
        out[kit] = np.where(absrel <= WIN, absrel, 1e9)
    return np.ascontiguousarray(out.transpose(1, 0, 2).reshape(128, NKT * QB))


def _numpy_reference(x, Wq, bq, Wk, bk, Wv, bv, Wo, bo, W1, b1, W2, b2,
                     g1, be1, g2, be2):
    from scipy.special import erf

    def ln(t, g, b):
        mu = t.mean(-1, keepdims=True)
        var = t.var(-1, keepdims=True)
        return (t - mu) / np.sqrt(var + EPS) * g + b

    Bv, Lv, Dv = x.shape
    pos = np.arange(Lv)
    rel = pos[:, None] - pos[None, :]
    mask = np.abs(rel) <= WIN
    slopes = np.asarray(_alibi_slopes(), dtype=np.float32)
    alibi = -slopes[:, None, None] * np.abs(rel)[None].astype(np.float32)
    q = (x @ Wq + bq).reshape(Bv, Lv, H, DH).transpose(0, 2, 1, 3)
    k = (x @ Wk + bk).reshape(Bv, Lv, H, DH).transpose(0, 2, 1, 3)
    v = (x @ Wv + bv).reshape(Bv, Lv, H, DH).transpose(0, 2, 1, 3)
    s = np.einsum("bhqd,bhkd->bhqk", q, k) / np.sqrt(np.float32(DH))
    s = s + alibi[None]
    s = np.where(mask[None, None], s, NEG)
    s = s - s.max(-1, keepdims=True)
    e = np.exp(s)
    attn = e / e.sum(-1, keepdims=True)
    ctx = np.einsum("bhqk,bhkd->bhqd", attn, v)
    ctx = ctx.transpose(0, 2, 1, 3).reshape(Bv, Lv, Dv)
    sa = ctx @ Wo + bo
    hh = ln(x + sa, g1, be1)
    ff = hh @ W1 + b1
    ff = ff * 0.5 * (1 + erf(ff / np.sqrt(2.0)))
    ff = ff @ W2 + b2
    return ln(hh + ff, g2, be2).astype(np.float32)


def kernel(**inputs):
    from ml_dtypes import bfloat16

    x = np.asarray(inputs["x"], dtype=np.float32)
    Wq = np.asarray(inputs["Wq"], dtype=np.float32)
    Wk = np.asarray(inputs["Wk"], dtype=np.float32)
    Wv = np.asarray(inputs["Wv"], dtype=np.float32)
    Wo = np.asarray(inputs["Wo"], dtype=np.float32)
    W1 = np.asarray(inputs["W1"], dtype=np.float32)
    W2 = np.asarray(inputs["W2"], dtype=np.float32)

    trivial_affine = all(
        np.all(np.asarray(inputs[n]) == 0)
        for n in ("bq", "bk", "bv", "bo", "b1", "b2", "be1", "be2")
    ) and all(np.all(np.asarray(inputs[n]) == 1) for n in ("g1", "g2"))
    if not trivial_affine:
        return _numpy_reference(
            x, Wq, inputs["bq"], Wk, inputs["bk"], Wv, inputs["bv"],
            Wo, inputs["bo"], W1, inputs["b1"], W2, inputs["b2"],
            inputs["g1"], inputs["be1"], inputs["g2"], inputs["be2"])

    if "nc" not in _NC_CACHE:
        _NC_CACHE["nc"] = _build_nc()
    nc = _NC_CACHE["nc"]

    base_ea = _make_base_ea().astype(bfloat16)
    wq_b = Wq.astype(bfloat16)
    wk_b = Wk.astype(bfloat16)
    wv_b = Wv.astype(bfloat16)
    wo_b = Wo.astype(bfloat16)
    w1_b = W1.astype(bfloat16)
    w2_b = W2.astype(bfloat16)

    in_maps = []
    for c in range(N_CORES):
        b = c // (N_CORES // B)
        l0 = (c % (N_CORES // B)) * CHUNK
        xpad = np.zeros((NKV, D), np.float32)
        lo, hi = l0 - WIN, l0 + CHUNK + WIN
        slo, shi = max(lo, 0), min(hi, L)
        xpad[slo - lo:shi - lo] = x[b, slo:shi]
        j = np.arange(NKV)
        valid = ((lo + j >= 0) & (lo + j < L)).astype(np.float32)  # [NKV]
        vmask = np.repeat(
            valid.reshape(NVT, 128).T[:, :, None], H, axis=2).reshape(
            128, NVT * H)
        in_maps.append({
            "xT": np.ascontiguousarray(xpad.T).astype(bfloat16),
            "x_own": np.ascontiguousarray(x[b, l0:l0 + CHUNK]).astype(bfloat16),
            "wq": wq_b, "wk": wk_b, "wv": wv_b, "wo": wo_b,
            "w1": w1_b, "w2": w2_b,
            "base_ea": base_ea,
            "vmask": vmask.astype(bfloat16),
        })

    _NC_CACHE["in_maps"] = in_maps
    res = run_bass_kernel_spmd(nc, in_maps, list(range(N_CORES)))
    out = np.empty((B, L, D), np.float32)
    for c in range(N_CORES):
        b = c // (N_CORES // B)
        l0 = (c % (N_CORES // B)) * CHUNK
        out[b, l0:l0 + CHUNK] = res.results[c]["out"]
    return out


# revision 25
# speedup vs baseline: 1.0604x; 1.0604x over previous
"""Trainium2 Bass kernel for nn_EncoderBlock (sliding-window attention + ALiBi
encoder block), SPMD over 8 NeuronCores.

Sharding: sequence-parallel. Token rows (B=2 x L=2048 = 4096) are split into 8
chunks of 512 (4 chunks per batch element). Each core computes its 512 output
rows end-to-end; the sliding window (|i-j| <= 64) only needs a 64-token K/V
halo on each side, so there are no collectives. Halo positions that fall
outside the sequence are zero-padded; masking works multiplicatively: the
attention probability is exp(score) * A where A_h = exp(-slope_h * |rel|)
(zero outside the window; computed on-chip from a shared |rel| base table with
one Exp per head), and the softmax denominator comes from a validity-masked
ones-column appended to V (pad keys contribute to neither numerator nor
denominator because their V rows and ones entries are zero).

Numerics: all matmul operands are bf16 (1 cycle/row on the PE, half the HBM
traffic of fp32), accumulating in fp32 PSUM. Softmax/LayerNorm statistics are
fp32. Residuals and intermediate activations are bf16.

Engine placement: PE does matmuls only (the softmax-denominator broadcast
runs on the Pool engine via partition_broadcast, LayerNorm runs on DVE+Pool),
weight streams live in dedicated never-released SBUF pools so their DMAs have
no write-after-read dependencies and prefetch under compute. w2's stream is
dispatched from the Pool queue so it cannot head-of-line block the sync queue.

NOTE: this kernel assumes the projection biases (bq,bk,bv,bo,b1,b2) are zero
and the LayerNorm affines are identity (g=1, be=0), which is what
setup_inputs() produces. It verifies this on the host and falls back to a
numpy reference implementation if violated.
"""

import math

import numpy as np

import concourse.bass as bass
import concourse.mybir as mybir
import concourse.tile as tile
from concourse import bacc
from concourse.bass_utils import run_bass_kernel_spmd
from concourse.masks import make_identity

F32 = mybir.dt.float32
BF16 = mybir.dt.bfloat16
AF = mybir.ActivationFunctionType
ALU = mybir.AluOpType
AX = mybir.AxisListType

B, L, D = 2, 2048, 1024
H, DH = 16, 64
FF = 4096
WIN = 64
NEG = -1e9
EPS = 1e-5
N_CORES = 8

CHUNK = (B * L) // N_CORES          # 512 own tokens per core
NKV = CHUNK + 2 * WIN               # 640 kv tokens (with halo)
QB = 256                            # query block (free dim of scores matmuls)
NQB = CHUNK // QB                   # 2 query blocks
NKT = (QB + 2 * WIN) // 128         # 3 key tiles of 128 per query block
DT = D // 128                       # 8 feature tiles
FT = FF // 128                      # 32 ff tiles
MT = CHUNK // 128                   # 4 token tiles
NVT = NKV // 128                    # 5 kv token tiles
VW = H * (DH + 1)                   # 1040: V row width incl. per-head ones col
W1G = 8                             # w1 stream groups (4 ff-tiles each)

_NC_CACHE = {}


def _alibi_slopes():
    return [2.0 ** (-8.0 * (h + 1) / H) for h in range(H)]


def _build_nc(loop=0, timing=False):
    nc = bacc.Bacc(None, target_bir_lowering=False)

    if timing:
        # weights live in Internal DRAM (garbage values) so the benchmark
        # upload is tiny.
        mk = lambda name, shape, dt: nc.dram_tensor(name, shape, dt).ap()
    else:
        mk = lambda name, shape, dt: nc.declare_dram_parameter(
            name, shape, dt, isOutput=False)
    xT = mk("xT", [D, NKV], BF16)
    x_own = mk("x_own", [CHUNK, D], BF16)
    wq = mk("wq", [D, D], BF16)
    wk = mk("wk", [D, D], BF16)
    wv = mk("wv", [D, D], BF16)
    wo = mk("wo", [D, D], BF16)
    w1 = mk("w1", [D, FF], BF16)
    w2 = mk("w2", [FF, D], BF16)
    # |rel| base table in scores^T layout [ki, kit*QB+qi]: |rel| in-window,
    # 1e9 outside (exp(-slope*1e9) == 0)
    base_ea = mk("base_ea", [128, NKT * QB], BF16)
    # per-(kv-token) validity for the softmax-denominator ones column,
    # expanded across heads: vmask[p, t*H + h] = 1.0 if kv token t*128+p is a
    # real (non-pad) position else 0.0
    vmask = nc.declare_dram_parameter("vmask", [128, NVT * H], BF16,
                                      isOutput=False)
    out = nc.declare_dram_parameter("out", [CHUNK, D], F32, isOutput=True)

    with nc.allow_low_precision(reason="bf16 matmul pipeline"), \
            tile.TileContext(nc) as tc:
        if loop:
            with tc.For_i(0, loop, 1):
                _body(nc, tc, xT, x_own, wq, wk, wv, wo, w1, w2,
                      base_ea, vmask, out)
        else:
            _body(nc, tc, xT, x_own, wq, wk, wv, wo, w1, w2, base_ea,
                  vmask, out)
    nc.finalize()
    return nc


def _body(nc, tc, xT, x_own, wq, wk, wv, wo, w1, w2, base_ea, vmask, out):
    P = lambda **kw: tc.alloc_tile_pool(**kw)
    # left-stack order: const, wstr, sm, ctxp, attp (top) -- attp released
    # after P2, ctxp after P3; const/wstr/sm live the whole kernel.
    const = P(name="const", bufs=1, side="left")
    wstr = P(name="wstream", bufs=1, side="left")
    sm = P(name="small", bufs=1, side="left")
    ctxp = P(name="ctxp", bufs=1, side="left")
    attp = P(name="attp", bufs=1, side="left")

    # ---- resident tiles + their DMAs (sync queue, in consumption order) ----
    xT_sb = attp.tile([128, DT * NKV], BF16, tag="xT")             # 10KB/part
    nc.sync.dma_start(
        out=xT_sb[:].rearrange("p (t c) -> p t c", t=DT),
        in_=xT.rearrange("(t p) c -> p t c", p=128))
    wq_sb = const.tile([128, DT * D], BF16, tag="wq")              # 16KB/part
    nc.sync.dma_start(
        out=wq_sb[:].rearrange("p (t c) -> p t c", t=DT),
        in_=wq.rearrange("(t p) c -> p t c", p=128))
    wk_sb = const.tile([128, DT * D], BF16, tag="wk")              # 16KB/part
    nc.sync.dma_start(
        out=wk_sb[:].rearrange("p (t c) -> p t c", t=DT),
        in_=wk.rearrange("(t p) c -> p t c", p=128))
    vm_sb = const.tile([128, NVT * H], BF16, tag="vmask")
    nc.sync.dma_start(out=vm_sb[:], in_=vmask[:])
    bea_sb = const.tile([128, NKT * QB], BF16, tag="base_ea")      # 1.5KB/part
    nc.sync.dma_start(out=bea_sb[:], in_=base_ea[:])
    wv_sb = const.tile([128, DT * D], BF16, tag="wv")              # 16KB/part
    nc.sync.dma_start(
        out=wv_sb[:].rearrange("p (t c) -> p t c", t=DT),
        in_=wv.rearrange("(t p) c -> p t c", p=128))
    wo_sb = const.tile([128, DT * D], BF16, tag="wo")              # 16KB/part
    nc.sync.dma_start(
        out=wo_sb[:].rearrange("p (t c) -> p t c", t=DT),
        in_=wo.rearrange("(t p) c -> p t c", p=128))
    xo_sb = ctxp.tile([128, MT * D], BF16, tag="x_own")            # 8KB/part
    nc.sync.dma_start(
        out=xo_sb[:].rearrange("p (t c) -> p t c", t=MT),
        in_=x_own.rearrange("(t p) c -> p t c", p=128))

    ident = sm.tile([128, 128], BF16, tag="ident")
    make_identity(nc, ident)
    ones_sb = sm.tile([1, 64], BF16, tag="ones")
    nc.vector.memset(ones_sb[:], 1.0)
    # per-head scaled identities for the fused-ALiBi matmul: -sqrt(DH)*slope_h
    # (the Exp's 1/sqrt(DH) input scale cancels it back to -slope_h)
    slopes = _alibi_slopes()
    identh = sm.tile([128, H * 128], BF16, tag="identh")           # 4KB/part
    for h in range(H):
        nc.scalar.mul(identh[:, h * 128:(h + 1) * 128], ident[:],
                      -math.sqrt(DH) * slopes[h])
    # w1 stream: 8 groups of 4 ff-tiles (512 ff cols), double buffered.
    # Fresh buffers -> the first group DMAs have no deps and prefetch early.
    w1_tiles = []
    for g in range(W1G):
        w1g = wstr.tile([128, DT * 512], BF16, tag="w1", bufs=2,
                        name=f"w1g{g}")                            # 8KB/part
        nc.scalar.dma_start(
            out=w1g[:].rearrange("p (t c) -> p t c", t=DT),
            in_=w1[:, g * 512:(g + 1) * 512].rearrange(
                "(t p) c -> p t c", p=128))
        w1_tiles.append(w1g)

    # ---- attention-phase pools --------------------------------------------
    ps_qkv = P(name="ps_qkv", bufs=1, space="PSUM")

    qT_sb = attp.tile([128, DT * CHUNK], BF16, tag="qT")           # 8KB/part
    kT_sb = attp.tile([128, DT * NKV], BF16, tag="kT")             # 10KB/part
    v_sb = attp.tile([128, NVT * VW], BF16, tag="v")               # 10.2KB/part
    ctxT_sb = ctxp.tile([128, DT * CHUNK], BF16, tag="ctxT")       # 8KB/part

    # ones-column of V' (softmax denominator), validity-masked for pad keys
    vo_ap = v_sb[:].rearrange("p (t h c) -> p t h c", t=NVT, h=H)
    nc.scalar.copy(vo_ap[:, :, :, 64],
                   vm_sb[:].rearrange("p (t h) -> p t h", t=NVT))

    # ---- P1: QKV projections ----------------------------------------------
    for do in range(DT):
        q_ps = ps_qkv.tile([128, CHUNK], F32, tag="qkv", bufs=3)
        for di in range(DT):
            nc.tensor.matmul(q_ps[:],
                             wq_sb[:, di * D + do * 128:di * D + (do + 1) * 128],
                             xT_sb[:, di * NKV + WIN:di * NKV + WIN + CHUNK],
                             start=(di == 0), stop=(di == DT - 1))
        nc.scalar.copy(qT_sb[:, do * CHUNK:(do + 1) * CHUNK], q_ps[:])
    for do in range(DT):
        for hf in range(2):
            k_ps = ps_qkv.tile([128, NKV // 2], F32, tag="qkv", bufs=3)
            for di in range(DT):
                nc.tensor.matmul(
                    k_ps[:],
                    wk_sb[:, di * D + do * 128:di * D + (do + 1) * 128],
                    xT_sb[:, di * NKV + hf * (NKV // 2):
                          di * NKV + (hf + 1) * (NKV // 2)],
                    start=(di == 0), stop=(di == DT - 1))
            nc.scalar.copy(
                kT_sb[:, do * NKV + hf * (NKV // 2):
                      do * NKV + (hf + 1) * (NKV // 2)], k_ps[:])
    for tt in range(NVT):
        for hf in range(2):
            v_ps = ps_qkv.tile([128, 512], F32, tag="qkv", bufs=3)
            for di in range(DT):
                nc.tensor.matmul(
                    v_ps[:],
                    xT_sb[:, di * NKV + tt * 128:di * NKV + (tt + 1) * 128],
                    wv_sb[:, di * D + hf * 512:di * D + (hf + 1) * 512],
                    start=(di == 0), stop=(di == DT - 1))
            # scatter heads: dout j -> col (h*65 + j%64), h = hf*8 + j//64
            dst = v_sb[:, tt * VW + hf * 8 * 65:tt * VW + (hf + 1) * 8 * 65]
            nc.scalar.copy(
                dst.rearrange("p (h c) -> p h c", h=8)[:, :, 0:64],
                v_ps[:].rearrange("p (h c) -> p h c", h=8))
    ps_qkv.release()

    # ---- P2: attention -----------------------------------------------------
    # Per head: scores for both query blocks land in one 3-bank PSUM tile;
    # ALiBi (+window mask) is accumulated INTO the scores by one extra matmul
    # (lhsT = -sqrt(DH)*slope_h * I, rhs = |rel| base table), so a single Exp
    # yields the unnormalized probabilities directly in bf16.
    ws2 = P(name="ws2", bufs=1, side="right")
    ps_att = P(name="ps_att", bufs=1, space="PSUM")
    inv_sqrt_dh = 1.0 / math.sqrt(DH)
    SW = NQB * NKT * QB                                  # 1536 score cols
    for h in range(H):
        hp = (h % 2) * 64
        dt_h = h // 2
        s_ps = ps_att.tile([128, SW], F32, tag="s", bufs=2)
        for qb in range(NQB):
            for kit in range(NKT):
                sl = s_ps[:, (qb * NKT + kit) * QB:(qb * NKT + kit + 1) * QB]
                # alibi first (starts the group), then scores accumulate on
                # top -- a canonical same-region 2-matmul accumulation group
                nc.tensor.matmul(
                    sl, identh[:, h * 128:(h + 1) * 128],
                    bea_sb[:, kit * QB:(kit + 1) * QB],
                    start=True, stop=False, skip_group_check=True)
                koff = dt_h * NKV + qb * QB + kit * 128
                nc.tensor.matmul(
                    sl, kT_sb[hp:hp + 64, koff:koff + 128],
                    qT_sb[hp:hp + 64, dt_h * CHUNK + qb * QB:
                          dt_h * CHUNK + (qb + 1) * QB],
                    start=False, stop=True, skip_group_check=True)
        pf = ws2.tile([128, SW], BF16, tag="pf", bufs=2)
        nc.scalar.activation(pf[:], s_ps[:], AF.Exp, scale=inv_sqrt_dh)
        c_ps = ps_att.tile([65, NQB * QB], F32, tag="ctx", bufs=1)
        for qb in range(NQB):
            for kit in range(NKT):
                vt = qb * 2 + kit
                nc.tensor.matmul(
                    c_ps[:, qb * QB:(qb + 1) * QB],
                    v_sb[:, vt * VW + h * 65:vt * VW + (h + 1) * 65],
                    pf[:, (qb * NKT + kit) * QB:(qb * NKT + kit + 1) * QB],
                    start=(kit == 0), stop=(kit == NKT - 1))
        ctx_sb = ws2.tile([65, NQB * QB], F32, tag="ctxe", bufs=2)
        nc.scalar.copy(ctx_sb[:], c_ps[:])
        rcf_sb = ws2.tile([1, NQB * QB], BF16, tag="rcf", bufs=2)
        nc.vector.reciprocal(rcf_sb[:], ctx_sb[64:65, :])
        b_ps = ps_att.tile([64, NQB * QB], F32, tag="bcast", bufs=1)
        nc.tensor.matmul(b_ps[:], ones_sb[:], rcf_sb[:], start=True, stop=True)
        nc.vector.tensor_tensor(
            out=ctxT_sb[hp:hp + 64, dt_h * CHUNK:(dt_h + 1) * CHUNK],
            in0=ctx_sb[0:64, :], in1=b_ps[:], op=ALU.mult)
    ws2.release()
    ps_att.release()
    attp.release()

    # ---- P3: Wo + residual + LN1, P4: transpose ---------------------------
    ffn = P(name="ffn", bufs=1, side="right")
    ps_wo = P(name="ps_wo", bufs=1, space="PSUM")
    h_sb = ffn.tile([128, MT * D], BF16, tag="h")                  # 8KB/part
    hT_sb = ffn.tile([128, DT * CHUNK], BF16, tag="hT")            # 8KB/part
    for m in range(MT):
        hpre = ffn.tile([128, D], BF16, tag="hpre", bufs=2)
        for nh in range(2):
            sa_ps = ps_wo.tile([128, 512], F32, tag="sa", bufs=2)
            for dt_ in range(DT):
                nc.tensor.matmul(
                    sa_ps[:],
                    ctxT_sb[:, dt_ * CHUNK + m * 128:dt_ * CHUNK + (m + 1) * 128],
                    wo_sb[:, dt_ * D + nh * 512:dt_ * D + (nh + 1) * 512],
                    start=(dt_ == 0), stop=(dt_ == DT - 1))
            nc.vector.tensor_tensor(
                out=hpre[:, nh * 512:(nh + 1) * 512], in0=sa_ps[:],
                in1=xo_sb[:, m * D + nh * 512:m * D + (nh + 1) * 512],
                op=ALU.add)
        _layernorm(nc, sm, ffn, hpre[:], h_sb[:, m * D:(m + 1) * D], m, "ln1")
        for dt_ in range(DT):
            t_ps = ps_wo.tile([128, 128], BF16, tag="tr", bufs=2)
            nc.tensor.transpose(
                t_ps[:], h_sb[:, m * D + dt_ * 128:m * D + (dt_ + 1) * 128],
                ident[:])
            nc.scalar.copy(
                hT_sb[:, dt_ * CHUNK + m * 128:dt_ * CHUNK + (m + 1) * 128],
                t_ps[:])
    ctxp.release()
    ps_wo.release()

    # ---- P5: fc1 + gelu ----------------------------------------------------
    ps_ffn = P(name="ps_ffn", bufs=1, space="PSUM")
    gT_sb = ffn.tile([128, FT * CHUNK], BF16, tag="gT")            # 32KB/part
    for g in range(W1G):
        for f4 in range(FT // W1G):
            ft = g * (FT // W1G) + f4
            f_ps = ps_ffn.tile([128, CHUNK], F32, tag="fc1", bufs=3)
            for di in range(DT):
                nc.tensor.matmul(
                    f_ps[:],
                    w1_tiles[g][:, di * 512 + f4 * 128:di * 512 + (f4 + 1) * 128],
                    hT_sb[:, di * CHUNK:(di + 1) * CHUNK],
                    start=(di == 0), stop=(di == DT - 1))
            nc.scalar.activation(gT_sb[:, ft * CHUNK:(ft + 1) * CHUNK],
                                 f_ps[:], AF.Gelu)

    # ---- P6: fc2 (w2 streamed on the Pool queue) + residual + LN2 ---------
    hpre2_tiles = [ffn.tile([128, D], BF16, tag="hpre2", bufs=MT,
                            name=f"hpre2_{m}") for m in range(MT)]
    for nh in range(2):
        o_ps_tiles = [ps_ffn.tile([128, 512], F32, tag=f"fc2_{m}", bufs=1,
                                  name=f"ops{nh}_{m}") for m in range(MT)]
        for kfg in range(FT // 4):
            w2g = wstr.tile([128, 4 * 512], BF16, tag="w2", bufs=2)
            nc.gpsimd.dma_start(
                out=w2g[:].rearrange("p (k c) -> p k c", k=4),
                in_=w2[kfg * 512:(kfg + 1) * 512,
                       nh * 512:(nh + 1) * 512].rearrange(
                    "(k p) c -> p k c", p=128))
            for k4 in range(4):
                kf = kfg * 4 + k4
                for m in range(MT):
                    nc.tensor.matmul(
                        o_ps_tiles[m][:],
                        gT_sb[:, kf * CHUNK + m * 128:kf * CHUNK + (m + 1) * 128],
                        w2g[:, k4 * 512:(k4 + 1) * 512],
                        start=(kf == 0), stop=(kf == FT - 1))
        for m in range(MT):
            nc.vector.tensor_tensor(
                out=hpre2_tiles[m][:, nh * 512:(nh + 1) * 512],
                in0=o_ps_tiles[m][:],
                in1=h_sb[:, m * D + nh * 512:m * D + (nh + 1) * 512],
                op=ALU.add)
    for m in range(MT):
        o_sb = ffn.tile([128, D], F32, tag="osb", bufs=2)
        _layernorm(nc, sm, ffn, hpre2_tiles[m][:], o_sb[:], m, "ln2")
        nc.gpsimd.dma_start(out=out[m * 128:(m + 1) * 128, :], in_=o_sb[:])
    ps_ffn.release()
    ffn.release()
    sm.release()
    wstr.release()
    const.release()


def _layernorm(nc, sm, ws, x_ap, out_ap, m, name):
    """out = (x - mean(x)) * rsqrt(var(x) + EPS) along the free dim (D).
    Stats on DVE, sqrt on Act ([128,1] only), final normalize on Pool."""
    s1 = sm.tile([128, 1], F32, tag=f"{name}_s1", bufs=2, name=f"{name}s1{m}")
    nc.vector.tensor_reduce(out=s1[:], in_=x_ap, axis=AX.X, op=ALU.add)
    sq = ws.tile([128, D], F32, tag="lnsq", bufs=2, name=f"{name}sq{m}")
    ssq = sm.tile([128, 1], F32, tag=f"{name}_ssq", bufs=2, name=f"{name}ssq{m}")
    nc.scalar.activation(sq[:], x_ap, AF.Square, accum_out=ssq[:])
    nm = sm.tile([128, 1], F32, tag=f"{name}_nm", bufs=2, name=f"{name}nm{m}")
    nc.vector.tensor_scalar_mul(nm[:], s1[:], -1.0 / D)
    m2 = sm.tile([128, 1], F32, tag=f"{name}_m2", bufs=2, name=f"{name}m2{m}")
    nc.vector.tensor_tensor(out=m2[:], in0=nm[:], in1=nm[:], op=ALU.mult)
    var = sm.tile([128, 1], F32, tag=f"{name}_var", bufs=2, name=f"{name}var{m}")
    nc.vector.tensor_scalar(var[:], ssq[:], 1.0 / D, EPS, ALU.mult, ALU.add)
    nc.vector.tensor_tensor(out=var[:], in0=var[:], in1=m2[:], op=ALU.subtract)
    sd = sm.tile([128, 1], F32, tag=f"{name}_sd", bufs=2, name=f"{name}sd{m}")
    nc.scalar.activation(sd[:], var[:], AF.Sqrt)
    r = sm.tile([128, 1], F32, tag=f"{name}_r", bufs=2, name=f"{name}r{m}")
    nc.vector.reciprocal(r[:], sd[:])
    nc.vector.tensor_scalar(out_ap, x_ap, nm[:], r[:], ALU.add, ALU.mult)


# ---------------------------------------------------------------------------
# host side
# ---------------------------------------------------------------------------

def _make_base_ea():
    """base[ki, kit*QB + qi] = |rel| if |rel| <= WIN else 1e9,
    rel = qi - (kit*128 + ki) + WIN  (scores^T layout [ki, qi])."""
    ki = np.arange(128)
    qi = np.arange(QB)
    out = np.zeros((NKT, 128, QB), dtype=np.float32)
    for kit in range(NKT):
        rel = qi[None, :] - (kit * 128 + ki)[:, None] + WIN   # [128, QB]
        absrel = np.abs(rel).astype(np.float32)
        out[kit] = np.where(absrel <= WIN, absrel, 1e9)
    return np.ascontiguousarray(out.transpose(1, 0, 2).reshape(128, NKT * QB))


def _numpy_reference(x, Wq, bq, Wk, bk, Wv, bv, Wo, bo, W1, b1, W2, b2,
                     g1, be1, g2, be2):
    from scipy.special import erf

    def ln(t, g, b):
        mu = t.mean(-1, keepdims=True)
        var = t.var(-1, keepdims=True)
        return (t - mu) / np.sqrt(var + EPS) * g + b

    Bv, Lv, Dv = x.shape
    pos = np.arange(Lv)
    rel = pos[:, None] - pos[None, :]
    mask = np.abs(rel) <= WIN
    slopes = np.asarray(_alibi_slopes(), dtype=np.float32)
    alibi = -slopes[:, None, None] * np.abs(rel)[None].astype(np.float32)
    q = (x @ Wq + bq).reshape(Bv, Lv, H, DH).transpose(0, 2, 1, 3)
    k = (x @ Wk + bk).reshape(Bv, Lv, H, DH).transpose(0, 2, 1, 3)
    v = (x @ Wv + bv).reshape(Bv, Lv, H, DH).transpose(0, 2, 1, 3)
    s = np.einsum("bhqd,bhkd->bhqk", q, k) / np.sqrt(np.float32(DH))
    s = s + alibi[None]
    s = np.where(mask[None, None], s, NEG)
    s = s - s.max(-1, keepdims=True)
    e = np.exp(s)
    attn = e / e.sum(-1, keepdims=True)
    ctx = np.einsum("bhqk,bhkd->bhqd", attn, v)
    ctx = ctx.transpose(0, 2, 1, 3).reshape(Bv, Lv, Dv)
    sa = ctx @ Wo + bo
    hh = ln(x + sa, g1, be1)
    ff = hh @ W1 + b1
    ff = ff * 0.5 * (1 + erf(ff / np.sqrt(2.0)))
    ff = ff @ W2 + b2
    return ln(hh + ff, g2, be2).astype(np.float32)


def kernel(**inputs):
    from ml_dtypes import bfloat16

    x = np.asarray(inputs["x"], dtype=np.float32)
    Wq = np.asarray(inputs["Wq"], dtype=np.float32)
    Wk = np.asarray(inputs["Wk"], dtype=np.float32)
    Wv = np.asarray(inputs["Wv"], dtype=np.float32)
    Wo = np.asarray(inputs["Wo"], dtype=np.float32)
    W1 = np.asarray(inputs["W1"], dtype=np.float32)
    W2 = np.asarray(inputs["W2"], dtype=np.float32)

    trivial_affine = all(
        np.all(np.asarray(inputs[n]) == 0)
        for n in ("bq", "bk", "bv", "bo", "b1", "b2", "be1", "be2")
    ) and all(np.all(np.asarray(inputs[n]) == 1) for n in ("g1", "g2"))
    if not trivial_affine:
        return _numpy_reference(
            x, Wq, inputs["bq"], Wk, inputs["bk"], Wv, inputs["bv"],
            Wo, inputs["bo"], W1, inputs["b1"], W2, inputs["b2"],
            inputs["g1"], inputs["be1"], inputs["g2"], inputs["be2"])

    if "nc" not in _NC_CACHE:
        _NC_CACHE["nc"] = _build_nc()
    nc = _NC_CACHE["nc"]

    base_ea = _make_base_ea().astype(bfloat16)
    wq_b = Wq.astype(bfloat16)
    wk_b = Wk.astype(bfloat16)
    wv_b = Wv.astype(bfloat16)
    wo_b = Wo.astype(bfloat16)
    w1_b = W1.astype(bfloat16)
    w2_b = W2.astype(bfloat16)

    in_maps = []
    for c in range(N_CORES):
        b = c // (N_CORES // B)
        l0 = (c % (N_CORES // B)) * CHUNK
        xpad = np.zeros((NKV, D), np.float32)
        lo, hi = l0 - WIN, l0 + CHUNK + WIN
        slo, shi = max(lo, 0), min(hi, L)
        xpad[slo - lo:shi - lo] = x[b, slo:shi]
        j = np.arange(NKV)
        valid = ((lo + j >= 0) & (lo + j < L)).astype(np.float32)  # [NKV]
        vmask = np.repeat(
            valid.reshape(NVT, 128).T[:, :, None], H, axis=2).reshape(
            128, NVT * H)
        in_maps.append({
            "xT": np.ascontiguousarray(xpad.T).astype(bfloat16),
            "x_own": np.ascontiguousarray(x[b, l0:l0 + CHUNK]).astype(bfloat16),
            "wq": wq_b, "wk": wk_b, "wv": wv_b, "wo": wo_b,
            "w1": w1_b, "w2": w2_b,
            "base_ea": base_ea,
            "vmask": vmask.astype(bfloat16),
        })

    _NC_CACHE["in_maps"] = in_maps
    res = run_bass_kernel_spmd(nc, in_maps, list(range(N_CORES)))
    out = np.empty((B, L, D), np.float32)
    for c in range(N_CORES):
        b = c // (N_CORES // B)
        l0 = (c % (N_CORES // B)) * CHUNK
        out[b, l0:l0 + CHUNK] = res.results[c]["out"]
    return out
